# revision 4
# baseline (speedup 1.0000x reference)
import functools
import os

import numpy as np

import concourse.bass as bass
import concourse.bacc as bacc
import concourse.tile as tile
from concourse import mybir
from concourse.bass_utils import run_bass_kernel_spmd

B = 8
N = 2048
NT = N // 128
LAT = 64
XF = LAT + 1
HID = 128
ODIM = 64
N_CORES = 8

F32 = mybir.dt.float32
BF16 = mybir.dt.bfloat16
Act = mybir.ActivationFunctionType


@functools.lru_cache(maxsize=2)
def _build_v2():
    nc = bacc.Bacc(None, target_bir_lowering=False, debug=False)

    ATP_d = nc.declare_dram_parameter("ATP", [4, 128, NT, 512], BF16,
                                      isOutput=False)
    XP_d = nc.declare_dram_parameter("XP", [128, NT * XF], BF16,
                                     isOutput=False)
    W0P_d = nc.declare_dram_parameter("W0P", [XF, HID], BF16, isOutput=False)
    W1_d = nc.declare_dram_parameter("W1", [HID, HID], BF16, isOutput=False)
    W2_d = nc.declare_dram_parameter("W2", [HID, HID], BF16, isOutput=False)
    WO_d = nc.declare_dram_parameter("WO", [HID, ODIM], BF16, isOutput=False)
    B1_d = nc.declare_dram_parameter("B1", [HID, 1], F32, isOutput=False)
    B2_d = nc.declare_dram_parameter("B2", [HID, 1], F32, isOutput=False)
    BO_d = nc.declare_dram_parameter("BO", [ODIM, 1], F32, isOutput=False)
    IDS_d = nc.declare_dram_parameter("IDS", [128, 128], BF16, isOutput=False)
    ID_d = nc.declare_dram_parameter("ID", [128, 128], F32, isOutput=False)
    MSKP_d = nc.declare_dram_parameter("MSKP", [128, NT], F32, isOutput=False)
    Y_d = nc.declare_dram_parameter("Y", [N, ODIM], F32, isOutput=True)
    Y3 = Y_d[:].rearrange("(t p) f -> p t f", p=128)

    with tile.TileContext(nc) as tc:
        with (
            tc.tile_pool(name="const", bufs=1) as constp,
            tc.tile_pool(name="at", bufs=1) as atp,
            tc.tile_pool(name="ht", bufs=2) as htp,
            tc.tile_pool(name="msg", bufs=2) as msgp,
            tc.tile_pool(name="msgt", bufs=2) as msgtp,
            tc.tile_pool(name="aht", bufs=2) as ahtsp,
            tc.tile_pool(name="xo", bufs=1) as xop,
            tc.tile_pool(name="ahtps", bufs=2, space=bass.MemorySpace.PSUM) as ahtpp,
            tc.tile_pool(name="aggp", bufs=4, space=bass.MemorySpace.PSUM) as aggp,
            tc.tile_pool(name="workp", bufs=2, space=bass.MemorySpace.PSUM) as workp,
        ):
            xp_t = constp.tile([128, NT * XF], BF16, tag="xp")
            nc.scalar.dma_start(xp_t[:], XP_d[:])
            w0p_t = constp.tile([XF, HID], BF16, tag="w0p")
            nc.scalar.dma_start(w0p_t[:], W0P_d[:])
            ident_b = constp.tile([128, 128], BF16, tag="idb")
            nc.scalar.dma_start(ident_b[:], IDS_d[:])
            w1_t = constp.tile([HID, HID], BF16, tag="w1")
            nc.scalar.dma_start(w1_t[:], W1_d[:])
            b1_t = constp.tile([HID, 1], F32, tag="b1")
            nc.scalar.dma_start(b1_t[:], B1_d[:])
            w2_t = constp.tile([HID, HID], BF16, tag="w2")
            nc.sync.dma_start(w2_t[:], W2_d[:])
            b2_t = constp.tile([HID, 1], F32, tag="b2")
            nc.sync.dma_start(b2_t[:], B2_d[:])
            wo_t = constp.tile([HID, ODIM], BF16, tag="wo")
            nc.sync.dma_start(wo_t[:], WO_d[:])
            bo_t = constp.tile([ODIM, 1], F32, tag="bo")
            nc.sync.dma_start(bo_t[:], BO_d[:])
            ident_f = constp.tile([128, 128], F32, tag="idf")
            nc.sync.dma_start(ident_f[:], ID_d[:])
            mskP = constp.tile([128, NT], F32, tag="mskP")
            nc.sync.dma_start(mskP[:], MSKP_d[:])

            at_t = atp.tile([128, 4 * NT * 512], BF16, tag="at")
            at5 = at_t[:].rearrange("p (i j c) -> p i j c", j=NT, c=512)
            for i in range(4):
                step = 4 if i == 0 else 8
                for j0 in range(0, NT, step):
                    nc.gpsimd.dma_start(
                        at5[:, i, j0 : j0 + step, :],
                        ATP_d[i, :, j0 : j0 + step, :],
                    )

            xp3 = xp_t[:].rearrange("p (t f) -> p t f", f=XF)
            hT1 = htp.tile([128, N], BF16, tag="ht", name="hT1")
            msg1 = msgp.tile([128, N], BF16, tag="msg", name="msg1")
            out_sb = xop.tile([128, NT * ODIM], F32, tag="xo", name="out_sb")

            def agg_mm(ap_ps, i, j, msg_nat, start, stop):
                nc.tensor.matmul(
                    ap_ps[0:HID, :],
                    msg_nat[:, j * 128 : (j + 1) * 128],
                    at5[:, i, j, :],
                    start=start,
                    stop=stop,
                )

            def emit_relu(l, i, ap_ps, hT_next):
                with nc.named_scope(f"relu{l}"):
                    dst = hT_next[:, i * 512 : (i + 1) * 512]
                    if i % 2 == 0:
                        nc.scalar.activation(dst, ap_ps[0:HID, :], Act.Relu)
                    else:
                        nc.vector.tensor_scalar_max(dst, ap_ps[0:HID, :], 0.0)

            msgT_tiles = {}

            def emit_msgT(lname, i, hT, w_t, b_col):
                with nc.named_scope(lname):
                    mp = workp.tile([128, 512], F32, tag="workp",
                                    name=f"{lname}_mp{i}")
                    nc.tensor.matmul(
                        mp[0:HID, :],
                        w_t[:],
                        hT[:, i * 512 : (i + 1) * 512],
                        start=True,
                        stop=True,
                    )
                    msgT = msgtp.tile([128, 512], BF16, tag="msgt",
                                      name=f"{lname}_msgT{i}")
                    if i % 2 == 0:
                        nc.scalar.activation(
                            msgT[:], mp[0:HID, :], Act.Identity, bias=b_col[:]
                        )
                    else:
                        nc.vector.tensor_scalar_add(
                            msgT[:], mp[0:HID, :], b_col[:]
                        )
                    msgT_tiles[(lname, i)] = msgT

            def emit_msg_tp(lname, i, msg_nat):
                with nc.named_scope(lname):
                    msgT = msgT_tiles.pop((lname, i))
                    ps = workp.tile([128, 512], BF16, tag="workp",
                                    name=f"{lname}_tp{i}")
                    for q in range(4):
                        nc.tensor.transpose(
                            ps[:, q * 128 : (q + 1) * 128],
                            msgT[:, q * 128 : (q + 1) * 128],
                            ident_b[:],
                        )
                    nc.vector.tensor_copy(
                        msg_nat[:, i * 512 : (i + 1) * 512], ps[:]
                    )

            projT_tiles = {}

            def emit_projT(i, hT):
                with nc.named_scope("proj"):
                    pp = workp.tile([128, 512], F32, tag="workp",
                                    name=f"pp{i}")
                    nc.tensor.matmul(
                        pp[0:ODIM, :],
                        wo_t[:],
                        hT[:, i * 512 : (i + 1) * 512],
                        start=True,
                        stop=True,
                    )
                    projT = msgtp.tile([128, 512], F32, tag="msgt",
                                       name=f"projT{i}")
                    if i % 2 == 0:
                        nc.scalar.activation(
                            projT[0:ODIM, :], pp[0:ODIM, :], Act.Identity,
                            bias=bo_t[:],
                        )
                    else:
                        nc.vector.tensor_scalar_add(
                            projT[0:ODIM, :], pp[0:ODIM, :], bo_t[:]
                        )
                    projT_tiles[i] = projT

            def emit_proj_out(i):
                with nc.named_scope("proj"):
                    projT = projT_tiles.pop(i)
                    ps = workp.tile([128, 512], F32, tag="workp",
                                    name=f"otp{i}")
                    for q in range(4):
                        nc.tensor.transpose(
                            ps[:, q * 64 : (q + 1) * 64],
                            projT[0:ODIM, q * 128 : (q + 1) * 128],
                            ident_f[0:ODIM, 0:ODIM],
                        )
                    for q in range(4):
                        j = 4 * i + q
                        if q % 2 == 0:
                            nc.scalar.activation(
                                out_sb[:, j * ODIM : (j + 1) * ODIM],
                                ps[:, q * 64 : (q + 1) * 64],
                                Act.Copy,
                                scale=mskP[:, j : j + 1],
                            )
                        else:
                            nc.vector.tensor_scalar_mul(
                                out_sb[:, j * ODIM : (j + 1) * ODIM],
                                ps[:, q * 64 : (q + 1) * 64],
                                mskP[:, j : j + 1],
                            )
                    nc.sync.dma_start(
                        Y3[:, 4 * i : 4 * i + 4, :],
                        out_sb[:].rearrange("p (t f) -> p t f", f=ODIM)[
                            :, 4 * i : 4 * i + 4, :
                        ],
                    )

            aps1 = {}
            msg2 = None
            hT2 = None
            for i in range(4):
                with nc.named_scope("agg0"):
                    aht_ps = ahtpp.tile([128, 512], F32, tag="ahtps",
                                        name=f"aht{i}")
                    for j in range(NT):
                        nc.tensor.matmul(
                            aht_ps[0:XF, :],
                            xp3[:, j, :],
                            at5[:, i, j, :],
                            start=(j == 0),
                            stop=(j == NT - 1),
                        )
                    aht_sb = ahtsp.tile([128, 512], BF16, tag="aht",
                                        name=f"ahts{i}")
                    if i % 2 == 0:
                        nc.scalar.activation(
                            aht_sb[0:XF, :], aht_ps[0:XF, :], Act.Copy
                        )
                    else:
                        nc.vector.tensor_copy(aht_sb[0:XF, :], aht_ps[0:XF, :])
                with nc.named_scope("post0"):
                    post_ps = workp.tile([128, 512], F32, tag="workp",
                                         name=f"post{i}")
                    nc.tensor.matmul(
                        post_ps[0:HID, :],
                        w0p_t[:],
                        aht_sb[0:XF, :],
                        start=True,
                        stop=True,
                    )
                emit_relu(0, i, post_ps, hT1)
                emit_msgT("msg1", i, hT1, w1_t, b1_t)
                emit_msg_tp("msg1", i, msg1)
                if i < 3:
                    with nc.named_scope("agg1"):
                        for ip in range(i + 1):
                            if ip not in aps1:
                                aps1[ip] = aggp.tile(
                                    [128, 512], F32, tag="agg",
                                    name=f"agg1_{ip}"
                                )
                            j0 = 0 if ip == i else 4 * i
                            for j in range(j0, 4 * i + 4):
                                agg_mm(aps1[ip], ip, j, msg1,
                                       start=(j == 0), stop=(j == NT - 1))
                else:
                    msg2 = msgp.tile([128, N], BF16, tag="msg", name="msg2")
                    hT2 = htp.tile([128, N], BF16, tag="ht", name="hT2")
                    for ip in range(4):
                        if ip not in aps1:
                            aps1[ip] = aggp.tile(
                                [128, 512], F32, tag="agg", name=f"agg1_{ip}"
                            )
                        j0 = 12 if ip < 3 else 0
                        with nc.named_scope("agg1"):
                            for j in range(j0, NT):
                                agg_mm(aps1[ip], ip, j, msg1,
                                       start=(j == 0), stop=(j == NT - 1))
                        emit_relu(1, ip, aps1[ip], hT2)
                        emit_msgT("msg2", ip, hT2, w2_t, b2_t)
                        if ip >= 1:
                            emit_msg_tp("msg2", ip - 1, msg2)
                    emit_msg_tp("msg2", 3, msg2)

            hT3 = htp.tile([128, N], BF16, tag="ht", name="hT3")
            for i in range(4):
                ap_ps = aggp.tile([128, 512], F32, tag="agg",
                                  name=f"agg2_{i}")
                with nc.named_scope("agg2"):
                    for j in range(NT):
                        agg_mm(ap_ps, i, j, msg2,
                               start=(j == 0), stop=(j == NT - 1))
                emit_relu(2, i, ap_ps, hT3)
                if i >= 1:
                    emit_projT(i - 1, hT3)
                if i >= 2:
                    emit_proj_out(i - 2)
            emit_proj_out(2)
            emit_projT(3, hT3)
            emit_proj_out(3)

    nc.compile()
    return nc


_EYE = np.eye(128, dtype=np.float32)


def _prep_v2(latent_features, adjacency_matrix, node_mask,
             W0, b0, W1, b1, W2, b2, Wout, bout):
    import ml_dtypes

    bf = ml_dtypes.bfloat16
    lat = np.asarray(latent_features, dtype=np.float32)
    adj = np.asarray(adjacency_matrix, dtype=np.float32)
    atp = np.ascontiguousarray(
        adj.reshape(B, 4, 512, NT, 128).transpose(0, 1, 4, 3, 2).astype(bf)
    )
    xa = np.concatenate(
        [lat, np.ones((B, N, 1), np.float32)], axis=2
    )
    xp = np.ascontiguousarray(
        xa.reshape(B, NT, 128, XF).transpose(0, 2, 1, 3).reshape(B, 128, NT * XF).astype(bf)
    )
    w0p = np.ascontiguousarray(
        np.concatenate(
            [np.asarray(W0, np.float32),
             np.asarray(b0, np.float32).reshape(1, HID)],
            axis=0,
        ).astype(bf)
    )
    w1b = np.ascontiguousarray(np.asarray(W1, np.float32).astype(bf))
    w2b = np.ascontiguousarray(np.asarray(W2, np.float32).astype(bf))
    wob = np.ascontiguousarray(np.asarray(Wout, np.float32).astype(bf))
    b1_ = np.asarray(b1, np.float32).reshape(HID, 1)
    b2_ = np.asarray(b2, np.float32).reshape(HID, 1)
    bo_ = np.asarray(bout, np.float32).reshape(ODIM, 1)
    msk = np.asarray(node_mask, np.float32).reshape(B, NT, 128)
    mskp = np.ascontiguousarray(msk.transpose(0, 2, 1))
    eye_b = np.ascontiguousarray(_EYE.astype(bf))

    in_maps = []
    for c in range(N_CORES):
        in_maps.append(
            {
                "ATP": atp[c],
                "XP": xp[c],
                "W0P": w0p,
                "W1": w1b,
                "B1": b1_,
                "W2": w2b,
                "B2": b2_,
                "WO": wob,
                "BO": bo_,
                "IDS": eye_b,
                "ID": _EYE,
                "MSKP": mskp[c],
            }
        )
    return in_maps


def kernel(
    latent_features,
    adjacency_matrix,
    node_mask,
    W0,
    b0,
    W1,
    b1,
    W2,
    b2,
    Wout,
    bout,
    _trace=False,
    _agg_dt=None,
):
    nc = _build_v2()
    in_maps = _prep_v2(latent_features, adjacency_matrix, node_mask,
                       W0, b0, W1, b1, W2, b2, Wout, bout)
    res = run_bass_kernel_spmd(
        nc, in_maps, core_ids=list(range(N_CORES)), trace=_trace
    )
    out = np.stack([res.results[c]["Y"] for c in range(N_CORES)], axis=0)
    if _trace:
        return out, res
    return out


# revision 10
# speedup vs baseline: 1.0962x; 1.0962x over previous
import functools
import os

import numpy as np

import concourse.bass as bass
import concourse.bacc as bacc
import concourse.tile as tile
from concourse import mybir
from concourse.bass_utils import run_bass_kernel_spmd

B = 8
N = 2048
NT = N // 128
LAT = 64
XF = LAT + 1
HID = 128
ODIM = 64
N_CORES = 8

F32 = mybir.dt.float32
BF16 = mybir.dt.bfloat16
Act = mybir.ActivationFunctionType


@functools.lru_cache(maxsize=2)
def _build_v2():
    nc = bacc.Bacc(None, target_bir_lowering=False, debug=False)

    ATP_d = nc.declare_dram_parameter("ATP", [4, 128, NT, 512], BF16,
                                      isOutput=False)
    XP_d = nc.declare_dram_parameter("XP", [128, NT * XF], BF16,
                                     isOutput=False)
    W0P_d = nc.declare_dram_parameter("W0P", [XF, HID], BF16, isOutput=False)
    W1_d = nc.declare_dram_parameter("W1", [HID, HID], BF16, isOutput=False)
    W2_d = nc.declare_dram_parameter("W2", [HID, HID], BF16, isOutput=False)
    WO_d = nc.declare_dram_parameter("WO", [HID, ODIM], BF16, isOutput=False)
    B1_d = nc.declare_dram_parameter("B1", [HID, 1], F32, isOutput=False)
    B2_d = nc.declare_dram_parameter("B2", [HID, 1], F32, isOutput=False)
    BO_d = nc.declare_dram_parameter("BO", [ODIM, 1], F32, isOutput=False)
    IDS_d = nc.declare_dram_parameter("IDS", [128, 128], BF16, isOutput=False)
    MSKP_d = nc.declare_dram_parameter("MSKP", [128, NT], F32, isOutput=False)
    Y_d = nc.declare_dram_parameter("Y", [N, ODIM], F32, isOutput=True)
    Y3 = Y_d[:].rearrange("(t p) f -> p t f", p=128)

    with tile.TileContext(nc) as tc:
        with (
            tc.tile_pool(name="const", bufs=1) as constp,
            tc.tile_pool(name="at", bufs=1) as atp,
            tc.tile_pool(name="ht", bufs=2) as htp,
            tc.tile_pool(name="msg", bufs=2) as msgp,
            tc.tile_pool(name="msgt", bufs=2) as msgtp,
            tc.tile_pool(name="aht", bufs=2) as ahtsp,
            tc.tile_pool(name="xo", bufs=1) as xop,
            tc.tile_pool(name="ahtps", bufs=2, space=bass.MemorySpace.PSUM) as ahtpp,
            tc.tile_pool(name="aggp", bufs=4, space=bass.MemorySpace.PSUM) as aggp,
            tc.tile_pool(name="workp", bufs=2, space=bass.MemorySpace.PSUM) as workp,
        ):
            xp_t = constp.tile([128, NT * XF], BF16, tag="xp")
            nc.scalar.dma_start(xp_t[:], XP_d[:])
            w0p_t = constp.tile([XF, HID], BF16, tag="w0p")
            nc.scalar.dma_start(w0p_t[:], W0P_d[:])
            ident_b = constp.tile([128, 128], BF16, tag="idb")
            nc.scalar.dma_start(ident_b[:], IDS_d[:])
            w1_t = constp.tile([HID, HID], BF16, tag="w1")
            nc.scalar.dma_start(w1_t[:], W1_d[:])
            b1_t = constp.tile([HID, 1], F32, tag="b1")
            nc.scalar.dma_start(b1_t[:], B1_d[:])
            w2_t = constp.tile([HID, HID], BF16, tag="w2")
            nc.sync.dma_start(w2_t[:], W2_d[:])
            b2_t = constp.tile([HID, 1], F32, tag="b2")
            nc.sync.dma_start(b2_t[:], B2_d[:])
            wo_t = constp.tile([HID, ODIM], BF16, tag="wo")
            nc.sync.dma_start(wo_t[:], WO_d[:])
            bo_t = constp.tile([ODIM, 1], F32, tag="bo")
            nc.sync.dma_start(bo_t[:], BO_d[:])
            mskP = constp.tile([128, NT], F32, tag="mskP")
            nc.sync.dma_start(mskP[:], MSKP_d[:])

            at_t = atp.tile([128, 4 * NT * 512], BF16, tag="at")
            at5 = at_t[:].rearrange("p (i j c) -> p i j c", j=NT, c=512)
            for i in range(2):
                for j0 in range(0, NT, 8):
                    nc.gpsimd.dma_start(
                        at5[:, i, j0 : j0 + 8, :],
                        ATP_d[i, :, j0 : j0 + 8, :],
                    )
            for i in range(2, 4):
                nc.sync.dma_start(at5[:, i, :, :], ATP_d[i, :, :, :])

            xp3 = xp_t[:].rearrange("p (t f) -> p t f", f=XF)
            hT1 = htp.tile([128, N], BF16, tag="ht", name="hT1")
            msg1 = msgp.tile([128, N], BF16, tag="msg", name="msg1")
            out_sb = xop.tile([128, NT * ODIM], F32, tag="xo", name="out_sb")

            def agg_mm(ap_ps, i, j, msg_nat, start, stop):
                nc.tensor.matmul(
                    ap_ps[0:HID, :],
                    msg_nat[:, j * 128 : (j + 1) * 128],
                    at5[:, i, j, :],
                    start=start,
                    stop=stop,
                )

            def emit_relu(l, i, ap_ps, hT_next):
                with nc.named_scope(f"relu{l}"):
                    dst = hT_next[:, i * 512 : (i + 1) * 512]
                    if i % 2 == 0:
                        nc.scalar.activation(dst, ap_ps[0:HID, :], Act.Relu)
                    else:
                        nc.vector.tensor_scalar_max(dst, ap_ps[0:HID, :], 0.0)

            msgT_tiles = {}

            def emit_msgT(lname, i, hT, w_t, b_col):
                with nc.named_scope(lname):
                    mp = workp.tile([128, 512], F32, tag="workp",
                                    name=f"{lname}_mp{i}")
                    nc.tensor.matmul(
                        mp[0:HID, :],
                        w_t[:],
                        hT[:, i * 512 : (i + 1) * 512],
                        start=True,
                        stop=True,
                    )
                    msgT = msgtp.tile([128, 512], BF16, tag="msgt",
                                      name=f"{lname}_msgT{i}")
                    if i % 2 == 0:
                        nc.scalar.activation(
                            msgT[:], mp[0:HID, :], Act.Identity, bias=b_col[:]
                        )
                    else:
                        nc.vector.tensor_scalar_add(
                            msgT[:], mp[0:HID, :], b_col[:]
                        )
                    msgT_tiles[(lname, i)] = msgT

            def emit_msg_tp(lname, i, msg_nat):
                with nc.named_scope(lname):
                    msgT = msgT_tiles.pop((lname, i))
                    ps = workp.tile([128, 512], BF16, tag="workp",
                                    name=f"{lname}_tp{i}")
                    for q in range(4):
                        nc.tensor.transpose(
                            ps[:, q * 128 : (q + 1) * 128],
                            msgT[:, q * 128 : (q + 1) * 128],
                            ident_b[:],
                        )
                    nc.vector.tensor_copy(
                        msg_nat[:, i * 512 : (i + 1) * 512], ps[:]
                    )

            projT_tiles = {}

            def emit_projT(i, hT):
                with nc.named_scope("proj"):
                    pp = workp.tile([128, 512], F32, tag="workp",
                                    name=f"pp{i}")
                    nc.tensor.matmul(
                        pp[0:ODIM, :],
                        wo_t[:],
                        hT[:, i * 512 : (i + 1) * 512],
                        start=True,
                        stop=True,
                    )
                    projT = msgtp.tile([128, 512], BF16, tag="msgt",
                                       name=f"projT{i}")
                    if i % 2 == 0:
                        nc.scalar.activation(
                            projT[0:ODIM, :], pp[0:ODIM, :], Act.Identity,
                            bias=bo_t[:],
                        )
                    else:
                        nc.vector.tensor_scalar_add(
                            projT[0:ODIM, :], pp[0:ODIM, :], bo_t[:]
                        )
                    projT_tiles[i] = projT

            def emit_proj_out(i):
                with nc.named_scope("proj"):
                    projT = projT_tiles.pop(i)
                    ps = workp.tile([128, 512], BF16, tag="workp",
                                    name=f"otp{i}")
                    for q in range(4):
                        nc.tensor.transpose(
                            ps[:, q * 64 : (q + 1) * 64],
                            projT[0:ODIM, q * 128 : (q + 1) * 128],
                            ident_b[0:ODIM, 0:ODIM],
                        )
                    for q in range(4):
                        j = 4 * i + q
                        if q % 2 == 0:
                            nc.scalar.activation(
                                out_sb[:, j * ODIM : (j + 1) * ODIM],
                                ps[:, q * 64 : (q + 1) * 64],
                                Act.Copy,
                                scale=mskP[:, j : j + 1],
                            )
                        else:
                            nc.vector.tensor_scalar_mul(
                                out_sb[:, j * ODIM : (j + 1) * ODIM],
                                ps[:, q * 64 : (q + 1) * 64],
                                mskP[:, j : j + 1],
                            )
                    nc.sync.dma_start(
                        Y3[:, 4 * i : 4 * i + 4, :],
                        out_sb[:].rearrange("p (t f) -> p t f", f=ODIM)[
                            :, 4 * i : 4 * i + 4, :
                        ],
                    )

            aps1 = {}
            done1 = {ip: set() for ip in range(4)}
            pending = []

            def agg1_step(ip, j):
                if ip not in aps1:
                    aps1[ip] = aggp.tile(
                        [128, 512], F32, tag="agg", name=f"agg1_{ip}"
                    )
                start = not done1[ip]
                done1[ip].add(j)
                agg_mm(aps1[ip], ip, j, msg1,
                       start=start, stop=(len(done1[ip]) == NT))

            def emit_partials(n, cur_panel):
                with nc.named_scope("agg1"):
                    k, emitted = 0, 0
                    while k < len(pending) and emitted < n:
                        ip, j = pending[k]
                        if ip <= cur_panel:
                            pending.pop(k)
                            agg1_step(ip, j)
                            emitted += 1
                        else:
                            k += 1

            msg2 = None
            hT2 = None
            for i in range(4):
                with nc.named_scope("agg0"):
                    aht_ps = ahtpp.tile([128, 512], F32, tag="ahtps",
                                        name=f"aht{i}")
                    for j in range(NT):
                        nc.tensor.matmul(
                            aht_ps[0:XF, :],
                            xp3[:, j, :],
                            at5[:, i, j, :],
                            start=(j == 0),
                            stop=(j == NT - 1),
                        )
                    aht_sb = ahtsp.tile([128, 512], BF16, tag="aht",
                                        name=f"ahts{i}")
                    if i % 2 == 0:
                        nc.scalar.activation(
                            aht_sb[0:XF, :], aht_ps[0:XF, :], Act.Copy
                        )
                    else:
                        nc.vector.tensor_copy(aht_sb[0:XF, :], aht_ps[0:XF, :])
                emit_partials(4, i)
                with nc.named_scope("post0"):
                    post_ps = workp.tile([128, 512], F32, tag="workp",
                                         name=f"post{i}")
                    nc.tensor.matmul(
                        post_ps[0:HID, :],
                        w0p_t[:],
                        aht_sb[0:XF, :],
                        start=True,
                        stop=True,
                    )
                emit_partials(3, i)
                emit_relu(0, i, post_ps, hT1)
                emit_partials(3, i)
                emit_msgT("msg1", i, hT1, w1_t, b1_t)
                emit_partials(3, i)
                emit_msg_tp("msg1", i, msg1)
                for j in range(4 * i, 4 * i + 4):
                    for ip in range(4):
                        pending.append((ip, j))
                if i == 3:
                    msg2 = msgp.tile([128, N], BF16, tag="msg", name="msg2")
                    hT2 = htp.tile([128, N], BF16, tag="ht", name="hT2")
                    for ip in range(4):
                        with nc.named_scope("agg1"):
                            for (ipp, j) in [p for p in pending
                                             if p[0] == ip]:
                                pending.remove((ipp, j))
                                agg1_step(ip, j)
                        emit_relu(1, ip, aps1[ip], hT2)
                        emit_msgT("msg2", ip, hT2, w2_t, b2_t)
                        if ip >= 1:
                            emit_msg_tp("msg2", ip - 1, msg2)
                    emit_msg_tp("msg2", 3, msg2)

            hT3 = htp.tile([128, N], BF16, tag="ht", name="hT3")
            for i in range(4):
                ap_ps = aggp.tile([128, 512], F32, tag="agg",
                                  name=f"agg2_{i}")
                with nc.named_scope("agg2"):
                    for j in range(NT):
                        agg_mm(ap_ps, i, j, msg2,
                               start=(j == 0), stop=(j == NT - 1))
                emit_relu(2, i, ap_ps, hT3)
                if i >= 1:
                    emit_projT(i - 1, hT3)
                if i >= 2:
                    emit_proj_out(i - 2)
            emit_projT(3, hT3)
            emit_proj_out(2)
            emit_proj_out(3)

    nc.compile()
    return nc


_EYE = np.eye(128, dtype=np.float32)


def _prep_v2(latent_features, adjacency_matrix, node_mask,
             W0, b0, W1, b1, W2, b2, Wout, bout):
    import ml_dtypes

    bf = ml_dtypes.bfloat16
    lat = np.asarray(latent_features, dtype=np.float32)
    adj = np.asarray(adjacency_matrix, dtype=np.float32)
    atp = np.ascontiguousarray(
        adj.reshape(B, 4, 512, NT, 128).transpose(0, 1, 4, 3, 2).astype(bf)
    )
    xa = np.concatenate(
        [lat, np.ones((B, N, 1), np.float32)], axis=2
    )
    xp = np.ascontiguousarray(
        xa.reshape(B, NT, 128, XF).transpose(0, 2, 1, 3).reshape(B, 128, NT * XF).astype(bf)
    )
    w0p = np.ascontiguousarray(
        np.concatenate(
            [np.asarray(W0, np.float32),
             np.asarray(b0, np.float32).reshape(1, HID)],
            axis=0,
        ).astype(bf)
    )
    w1b = np.ascontiguousarray(np.asarray(W1, np.float32).astype(bf))
    w2b = np.ascontiguousarray(np.asarray(W2, np.float32).astype(bf))
    wob = np.ascontiguousarray(np.asarray(Wout, np.float32).astype(bf))
    b1_ = np.asarray(b1, np.float32).reshape(HID, 1)
    b2_ = np.asarray(b2, np.float32).reshape(HID, 1)
    bo_ = np.asarray(bout, np.float32).reshape(ODIM, 1)
    msk = np.asarray(node_mask, np.float32).reshape(B, NT, 128)
    mskp = np.ascontiguousarray(msk.transpose(0, 2, 1))
    eye_b = np.ascontiguousarray(_EYE.astype(bf))

    in_maps = []
    for c in range(N_CORES):
        in_maps.append(
            {
                "ATP": atp[c],
                "XP": xp[c],
                "W0P": w0p,
                "W1": w1b,
                "B1": b1_,
                "W2": w2b,
                "B2": b2_,
                "WO": wob,
                "BO": bo_,
                "IDS": eye_b,
                "MSKP": mskp[c],
            }
        )
    return in_maps


def kernel(
    latent_features,
    adjacency_matrix,
    node_mask,
    W0,
    b0,
    W1,
    b1,
    W2,
    b2,
    Wout,
    bout,
    _trace=False,
    _agg_dt=None,
):
    nc = _build_v2()
    in_maps = _prep_v2(latent_features, adjacency_matrix, node_mask,
                       W0, b0, W1, b1, W2, b2, Wout, bout)
    res = run_bass_kernel_spmd(
        nc, in_maps, core_ids=list(range(N_CORES)), trace=_trace
    )
    out = np.stack([res.results[c]["Y"] for c in range(N_CORES)], axis=0)
    if _trace:
        return out, res
    return out


# revision 15
# speedup vs baseline: 1.1507x; 1.0497x over previous
import functools
import os

import numpy as np

import concourse.bass as bass
import concourse.bacc as bacc
import concourse.tile as tile
from concourse import mybir
from concourse.bass_utils import run_bass_kernel_spmd

B = 8
N = 2048
NT = N // 128
LAT = 64
XF = LAT + 1
HID = 128
ODIM = 64
N_CORES = 8

F32 = mybir.dt.float32
BF16 = mybir.dt.bfloat16
Act = mybir.ActivationFunctionType


@functools.lru_cache(maxsize=2)
def _build_v2():
    nc = bacc.Bacc(None, target_bir_lowering=False, debug=False)

    ATP_d = nc.declare_dram_parameter("ATP", [4, 128, NT, 512], BF16,
                                      isOutput=False)
    XP_d = nc.declare_dram_parameter("XP", [128, NT * XF], BF16,
                                     isOutput=False)
    W0P_d = nc.declare_dram_parameter("W0P", [XF, HID], BF16, isOutput=False)
    W1_d = nc.declare_dram_parameter("W1", [HID, HID], BF16, isOutput=False)
    W2_d = nc.declare_dram_parameter("W2", [HID, HID], BF16, isOutput=False)
    WO_d = nc.declare_dram_parameter("WO", [HID, ODIM], BF16, isOutput=False)
    B1_d = nc.declare_dram_parameter("B1", [HID, 1], F32, isOutput=False)
    B2_d = nc.declare_dram_parameter("B2", [HID, 1], F32, isOutput=False)
    BO_d = nc.declare_dram_parameter("BO", [ODIM, 1], F32, isOutput=False)
    IDS_d = nc.declare_dram_parameter("IDS", [128, 128], BF16, isOutput=False)
    MSKP_d = nc.declare_dram_parameter("MSKP", [128, NT], F32, isOutput=False)
    Y_d = nc.declare_dram_parameter("Y", [N, ODIM], F32, isOutput=True)
    Y3 = Y_d[:].rearrange("(t p) f -> p t f", p=128)

    with tile.TileContext(nc) as tc:
        with (
            tc.tile_pool(name="const", bufs=1) as constp,
            tc.tile_pool(name="at", bufs=1) as atp,
            tc.tile_pool(name="ht", bufs=2) as htp,
            tc.tile_pool(name="msg", bufs=2) as msgp,
            tc.tile_pool(name="msgt", bufs=2) as msgtp,
            tc.tile_pool(name="aht", bufs=2) as ahtsp,
            tc.tile_pool(name="xo", bufs=1) as xop,
            tc.tile_pool(name="ahtps", bufs=1, space=bass.MemorySpace.PSUM) as ahtpp,
            tc.tile_pool(name="aggp", bufs=4, space=bass.MemorySpace.PSUM) as aggp,
            tc.tile_pool(name="workp", bufs=3, space=bass.MemorySpace.PSUM) as workp,
        ):
            xp_t = constp.tile([128, NT * XF], BF16, tag="xp")
            nc.scalar.dma_start(xp_t[:], XP_d[:])
            w0p_t = constp.tile([XF, HID], BF16, tag="w0p")
            nc.scalar.dma_start(w0p_t[:], W0P_d[:])
            ident_b = constp.tile([128, 128], BF16, tag="idb")
            nc.scalar.dma_start(ident_b[:], IDS_d[:])
            w1_t = constp.tile([HID, HID], BF16, tag="w1")
            nc.scalar.dma_start(w1_t[:], W1_d[:])
            b1_t = constp.tile([HID, 1], F32, tag="b1")
            nc.scalar.dma_start(b1_t[:], B1_d[:])
            w2_t = constp.tile([HID, HID], BF16, tag="w2")
            nc.sync.dma_start(w2_t[:], W2_d[:])
            b2_t = constp.tile([HID, 1], F32, tag="b2")
            nc.sync.dma_start(b2_t[:], B2_d[:])
            wo_t = constp.tile([HID, ODIM], BF16, tag="wo")
            nc.sync.dma_start(wo_t[:], WO_d[:])
            bo_t = constp.tile([ODIM, 1], F32, tag="bo")
            nc.sync.dma_start(bo_t[:], BO_d[:])
            mskP = constp.tile([128, NT], F32, tag="mskP")
            nc.sync.dma_start(mskP[:], MSKP_d[:])

            at_t = atp.tile([128, 4 * NT * 512], BF16, tag="at")
            at5 = at_t[:].rearrange("p (i j c) -> p i j c", j=NT, c=512)
            for i in range(4):
                step = 4 if i == 0 else 8
                for j0 in range(0, NT, step):
                    nc.gpsimd.dma_start(
                        at5[:, i, j0 : j0 + step, :],
                        ATP_d[i, :, j0 : j0 + step, :],
                    )

            xp3 = xp_t[:].rearrange("p (t f) -> p t f", f=XF)
            hT1 = htp.tile([128, N], BF16, tag="ht", name="hT1")
            msg1 = msgp.tile([128, N], BF16, tag="msg", name="msg1")
            out_sb = xop.tile([128, NT * ODIM], F32, tag="xo", name="out_sb")

            def agg_mm(ap_ps, i, j, msg_nat, start, stop):
                nc.tensor.matmul(
                    ap_ps[0:HID, :],
                    msg_nat[:, j * 128 : (j + 1) * 128],
                    at5[:, i, j, :],
                    start=start,
                    stop=stop,
                )

            def emit_relu(l, i, ap_ps, hT_next):
                with nc.named_scope(f"relu{l}"):
                    dst = hT_next[:, i * 512 : (i + 1) * 512]
                    if i % 2 == 0:
                        nc.scalar.activation(dst, ap_ps[0:HID, :], Act.Relu)
                    else:
                        nc.vector.tensor_scalar_max(dst, ap_ps[0:HID, :], 0.0)

            msgT_tiles = {}

            def emit_msgT(lname, i, hT, w_t, b_col):
                with nc.named_scope(lname):
                    mp = workp.tile([128, 512], F32, tag="workp",
                                    name=f"{lname}_mp{i}")
                    nc.tensor.matmul(
                        mp[0:HID, :],
                        w_t[:],
                        hT[:, i * 512 : (i + 1) * 512],
                        start=True,
                        stop=True,
                    )
                    msgT = msgtp.tile([128, 512], BF16, tag="msgt",
                                      name=f"{lname}_msgT{i}")
                    if i % 2 == 0:
                        nc.scalar.activation(
                            msgT[:], mp[0:HID, :], Act.Identity, bias=b_col[:]
                        )
                    else:
                        nc.vector.tensor_scalar_add(
                            msgT[:], mp[0:HID, :], b_col[:]
                        )
                    msgT_tiles[(lname, i)] = msgT

            def emit_msg_tp(lname, i, msg_nat):
                with nc.named_scope(lname):
                    msgT = msgT_tiles.pop((lname, i))
                    ps = workp.tile([128, 512], BF16, tag="workp",
                                    name=f"{lname}_tp{i}")
                    for q in range(4):
                        nc.tensor.transpose(
                            ps[:, q * 128 : (q + 1) * 128],
                            msgT[:, q * 128 : (q + 1) * 128],
                            ident_b[:],
                        )
                    nc.vector.tensor_copy(
                        msg_nat[:, i * 512 : (i + 1) * 512], ps[:]
                    )

            projT_tiles = {}

            def emit_projT(i, hT):
                with nc.named_scope("proj"):
                    pp = workp.tile([128, 512], F32, tag="workp",
                                    name=f"pp{i}")
                    nc.tensor.matmul(
                        pp[0:ODIM, :],
                        wo_t[:],
                        hT[:, i * 512 : (i + 1) * 512],
                        start=True,
                        stop=True,
                    )
                    projT = msgtp.tile([128, 512], BF16, tag="msgt",
                                       name=f"projT{i}")
                    if i % 2 == 0:
                        nc.scalar.activation(
                            projT[0:ODIM, :], pp[0:ODIM, :], Act.Identity,
                            bias=bo_t[:],
                        )
                    else:
                        nc.vector.tensor_scalar_add(
                            projT[0:ODIM, :], pp[0:ODIM, :], bo_t[:]
                        )
                    projT_tiles[i] = projT

            def emit_proj_out(i):
                with nc.named_scope("proj"):
                    projT = projT_tiles.pop(i)
                    ps = workp.tile([128, 512], BF16, tag="workp",
                                    name=f"otp{i}")
                    for q in range(4):
                        nc.tensor.transpose(
                            ps[:, q * 64 : (q + 1) * 64],
                            projT[0:ODIM, q * 128 : (q + 1) * 128],
                            ident_b[0:ODIM, 0:ODIM],
                        )
                    for q in range(4):
                        j = 4 * i + q
                        if q % 2 == 0:
                            nc.scalar.activation(
                                out_sb[:, j * ODIM : (j + 1) * ODIM],
                                ps[:, q * 64 : (q + 1) * 64],
                                Act.Copy,
                                scale=mskP[:, j : j + 1],
                            )
                        else:
                            nc.vector.tensor_scalar_mul(
                                out_sb[:, j * ODIM : (j + 1) * ODIM],
                                ps[:, q * 64 : (q + 1) * 64],
                                mskP[:, j : j + 1],
                            )
                    nc.sync.dma_start(
                        Y3[:, 4 * i : 4 * i + 4, :],
                        out_sb[:].rearrange("p (t f) -> p t f", f=ODIM)[
                            :, 4 * i : 4 * i + 4, :
                        ],
                    )

            aps1 = {}
            done1 = {ip: set() for ip in range(4)}
            pending = []

            def agg1_step(ip, j):
                if ip not in aps1:
                    aps1[ip] = aggp.tile(
                        [128, 512], F32, tag="agg", name=f"agg1_{ip}"
                    )
                start = not done1[ip]
                done1[ip].add(j)
                agg_mm(aps1[ip], ip, j, msg1,
                       start=start, stop=(len(done1[ip]) == NT))

            def emit_partials(n, max_ip):
                with nc.named_scope("agg1"):
                    k, emitted = 0, 0
                    while k < len(pending) and emitted < n:
                        ip, j = pending[k]
                        if ip <= max_ip:
                            pending.pop(k)
                            agg1_step(ip, j)
                            emitted += 1
                        else:
                            k += 1

            msg2 = None
            hT2 = None
            for i in range(4):
                emit_partials(99, i - 1)
                with nc.named_scope("agg0"):
                    aht_ps = ahtpp.tile([128, 512], F32, tag="ahtps",
                                        name=f"aht{i}")
                    for j in range(NT):
                        nc.tensor.matmul(
                            aht_ps[0:XF, :],
                            xp3[:, j, :],
                            at5[:, i, j, :],
                            start=(j == 0),
                            stop=(j == NT - 1),
                        )
                    aht_sb = ahtsp.tile([128, 512], BF16, tag="aht",
                                        name=f"ahts{i}")
                    if i % 2 == 0:
                        nc.scalar.activation(
                            aht_sb[0:XF, :], aht_ps[0:XF, :], Act.Copy
                        )
                    else:
                        nc.vector.tensor_copy(aht_sb[0:XF, :], aht_ps[0:XF, :])
                emit_partials(4, i)
                with nc.named_scope("post0"):
                    post_ps = workp.tile([128, 512], F32, tag="workp",
                                         name=f"post{i}")
                    nc.tensor.matmul(
                        post_ps[0:HID, :],
                        w0p_t[:],
                        aht_sb[0:XF, :],
                        start=True,
                        stop=True,
                    )
                emit_partials(3, i)
                emit_relu(0, i, post_ps, hT1)
                emit_partials(3, i)
                emit_msgT("msg1", i, hT1, w1_t, b1_t)
                emit_partials(3, i)
                emit_msg_tp("msg1", i, msg1)
                for j in range(4 * i, 4 * i + 4):
                    for ip in range(4):
                        pending.append((ip, j))
                if i == 3:
                    msg2 = msgp.tile([128, N], BF16, tag="msg", name="msg2")
                    hT2 = htp.tile([128, N], BF16, tag="ht", name="hT2")
                    for ip in range(4):
                        with nc.named_scope("agg1"):
                            for (ipp, j) in [p for p in pending
                                             if p[0] == ip]:
                                pending.remove((ipp, j))
                                agg1_step(ip, j)
                        emit_relu(1, ip, aps1[ip], hT2)
                        emit_msgT("msg2", ip, hT2, w2_t, b2_t)
                        if ip >= 1:
                            emit_msg_tp("msg2", ip - 1, msg2)
                    emit_msg_tp("msg2", 3, msg2)

            hT3 = htp.tile([128, N], BF16, tag="ht", name="hT3")
            for i in range(4):
                ap_ps = aggp.tile([128, 512], F32, tag="agg",
                                  name=f"agg2_{i}")
                with nc.named_scope("agg2"):
                    for j in range(NT):
                        agg_mm(ap_ps, i, j, msg2,
                               start=(j == 0), stop=(j == NT - 1))
                if i < 3:
                    emit_relu(2, i, ap_ps, hT3)
                    if i >= 1:
                        emit_projT(i - 1, hT3)
                    if i >= 2:
                        emit_proj_out(i - 2)
            with nc.named_scope("relu2"):
                nc.scalar.activation(
                    hT3[:, 1536:1792], ap_ps[0:HID, 0:256], Act.Relu
                )
                nc.vector.tensor_scalar_max(
                    hT3[:, 1792:2048], ap_ps[0:HID, 256:512], 0.0
                )
            emit_projT(2, hT3)
            emit_proj_out(1)
            projT3 = msgtp.tile([128, 512], BF16, tag="msgt", name="projT3")
            with nc.named_scope("proj"):
                for h in range(2):
                    c0 = 1536 + h * 256
                    pp = workp.tile([128, 512], F32, tag="workp",
                                    name=f"pp3h{h}")
                    nc.tensor.matmul(
                        pp[0:ODIM, 0:256],
                        wo_t[:],
                        hT3[:, c0 : c0 + 256],
                        start=True,
                        stop=True,
                    )
                    if h == 0:
                        nc.scalar.activation(
                            projT3[0:ODIM, 0:256], pp[0:ODIM, 0:256],
                            Act.Identity, bias=bo_t[:],
                        )
                        emit_proj_out(2)
                    else:
                        nc.vector.tensor_scalar_add(
                            projT3[0:ODIM, 256:512], pp[0:ODIM, 0:256],
                            bo_t[:],
                        )
                for h in range(2):
                    ps = workp.tile([128, 512], BF16, tag="workp",
                                    name=f"otp3h{h}")
                    for qq in range(2):
                        q = 2 * h + qq
                        nc.tensor.transpose(
                            ps[:, q * 64 : (q + 1) * 64],
                            projT3[0:ODIM, q * 128 : (q + 1) * 128],
                            ident_b[0:ODIM, 0:ODIM],
                        )
                    for qq in range(2):
                        q = 2 * h + qq
                        j = 12 + q
                        if qq == 0:
                            nc.scalar.activation(
                                out_sb[:, j * ODIM : (j + 1) * ODIM],
                                ps[:, q * 64 : (q + 1) * 64],
                                Act.Copy,
                                scale=mskP[:, j : j + 1],
                            )
                        else:
                            nc.vector.tensor_scalar_mul(
                                out_sb[:, j * ODIM : (j + 1) * ODIM],
                                ps[:, q * 64 : (q + 1) * 64],
                                mskP[:, j : j + 1],
                            )
                    nc.sync.dma_start(
                        Y3[:, 12 + 2 * h : 14 + 2 * h, :],
                        out_sb[:].rearrange("p (t f) -> p t f", f=ODIM)[
                            :, 12 + 2 * h : 14 + 2 * h, :
                        ],
                    )

    nc.compile()
    return nc


_EYE = np.eye(128, dtype=np.float32)


def _prep_v2(latent_features, adjacency_matrix, node_mask,
             W0, b0, W1, b1, W2, b2, Wout, bout):
    import ml_dtypes

    bf = ml_dtypes.bfloat16
    lat = np.asarray(latent_features, dtype=np.float32)
    adj = np.asarray(adjacency_matrix, dtype=np.float32)
    atp = np.ascontiguousarray(
        adj.reshape(B, 4, 512, NT, 128).transpose(0, 1, 4, 3, 2).astype(bf)
    )
    xa = np.concatenate(
        [lat, np.ones((B, N, 1), np.float32)], axis=2
    )
    xp = np.ascontiguousarray(
        xa.reshape(B, NT, 128, XF).transpose(0, 2, 1, 3).reshape(B, 128, NT * XF).astype(bf)
    )
    w0p = np.ascontiguousarray(
        np.concatenate(
            [np.asarray(W0, np.float32),
             np.asarray(b0, np.float32).reshape(1, HID)],
            axis=0,
        ).astype(bf)
    )
    w1b = np.ascontiguousarray(np.asarray(W1, np.float32).astype(bf))
    w2b = np.ascontiguousarray(np.asarray(W2, np.float32).astype(bf))
    wob = np.ascontiguousarray(np.asarray(Wout, np.float32).astype(bf))
    b1_ = np.asarray(b1, np.float32).reshape(HID, 1)
    b2_ = np.asarray(b2, np.float32).reshape(HID, 1)
    bo_ = np.asarray(bout, np.float32).reshape(ODIM, 1)
    msk = np.asarray(node_mask, np.float32).reshape(B, NT, 128)
    mskp = np.ascontiguousarray(msk.transpose(0, 2, 1))
    eye_b = np.ascontiguousarray(_EYE.astype(bf))

    in_maps = []
    for c in range(N_CORES):
        in_maps.append(
            {
                "ATP": atp[c],
                "XP": xp[c],
                "W0P": w0p,
                "W1": w1b,
                "B1": b1_,
                "W2": w2b,
                "B2": b2_,
                "WO": wob,
                "BO": bo_,
                "IDS": eye_b,
                "MSKP": mskp[c],
            }
        )
    return in_maps


def kernel(
    latent_features,
    adjacency_matrix,
    node_mask,
    W0,
    b0,
    W1,
    b1,
    W2,
    b2,
    Wout,
    bout,
    _trace=False,
    _agg_dt=None,
):
    nc = _build_v2()
    in_maps = _prep_v2(latent_features, adjacency_matrix, node_mask,
                       W0, b0, W1, b1, W2, b2, Wout, bout)
    res = run_bass_kernel_spmd(
        nc, in_maps, core_ids=list(range(N_CORES)), trace=_trace
    )
    out = np.stack([res.results[c]["Y"] for c in range(N_CORES)], axis=0)
    if _trace:
        return out, res
    return out


# revision 25
# speedup vs baseline: 1.2312x; 1.0699x over previous
import functools
import os

import numpy as np

import concourse.bass as bass
import concourse.bacc as bacc
import concourse.tile as tile
from concourse import mybir
from concourse.bass_utils import run_bass_kernel_spmd

B = 8
N = 2048
NT = N // 128
LAT = 64
XF = LAT + 1
HID = 128
ODIM = 64
N_CORES = 8

F32 = mybir.dt.float32
BF16 = mybir.dt.bfloat16
Act = mybir.ActivationFunctionType


@functools.lru_cache(maxsize=2)
def _build_v2():
    nc = bacc.Bacc(None, target_bir_lowering=False, debug=False)

    ATP_d = nc.declare_dram_parameter("ATP", [4, 128, NT, 512], BF16,
                                      isOutput=False)
    XP_d = nc.declare_dram_parameter("XP", [128, NT * XF], BF16,
                                     isOutput=False)
    W0P_d = nc.declare_dram_parameter("W0P", [XF, HID], BF16, isOutput=False)
    W1_d = nc.declare_dram_parameter("W1", [HID, HID], BF16, isOutput=False)
    W2_d = nc.declare_dram_parameter("W2", [HID, HID], BF16, isOutput=False)
    WO_d = nc.declare_dram_parameter("WO", [HID, ODIM], BF16, isOutput=False)
    B1_d = nc.declare_dram_parameter("B1", [128, HID], F32, isOutput=False)
    B2_d = nc.declare_dram_parameter("B2", [128, HID], F32, isOutput=False)
    BO_d = nc.declare_dram_parameter("BO", [ODIM, 1], F32, isOutput=False)
    IDS_d = nc.declare_dram_parameter("IDS", [128, 128], BF16, isOutput=False)
    MSKP_d = nc.declare_dram_parameter("MSKP", [128, NT], F32, isOutput=False)
    Y_d = nc.declare_dram_parameter("Y", [N, ODIM], F32, isOutput=True)
    Y3 = Y_d[:].rearrange("(t p) f -> p t f", p=128)

    with tile.TileContext(nc) as tc:
        with (
            tc.tile_pool(name="const", bufs=1) as constp,
            tc.tile_pool(name="at", bufs=1) as atp,
            tc.tile_pool(name="ht", bufs=2) as htp,
            tc.tile_pool(name="msg", bufs=2) as msgp,
            tc.tile_pool(name="msgt", bufs=2) as msgtp,
            tc.tile_pool(name="aht", bufs=2) as ahtsp,
            tc.tile_pool(name="xo", bufs=1) as xop,
            tc.tile_pool(name="ahtps", bufs=1, space=bass.MemorySpace.PSUM) as ahtpp,
            tc.tile_pool(name="aggp", bufs=4, space=bass.MemorySpace.PSUM) as aggp,
            tc.tile_pool(name="workp", bufs=3, space=bass.MemorySpace.PSUM) as workp,
        ):
            xp_t = constp.tile([128, NT * XF], BF16, tag="xp")
            nc.scalar.dma_start(xp_t[:], XP_d[:])
            w0p_t = constp.tile([XF, HID], BF16, tag="w0p")
            nc.scalar.dma_start(w0p_t[:], W0P_d[:])
            ident_b = constp.tile([128, 128], BF16, tag="idb")
            nc.scalar.dma_start(ident_b[:], IDS_d[:])
            w1_t = constp.tile([HID, HID], BF16, tag="w1")
            nc.scalar.dma_start(w1_t[:], W1_d[:])
            b1_t = constp.tile([128, HID], F32, tag="b1")
            nc.scalar.dma_start(b1_t[:], B1_d[:])
            w2_t = constp.tile([HID, HID], BF16, tag="w2")
            nc.sync.dma_start(w2_t[:], W2_d[:])
            b2_t = constp.tile([128, HID], F32, tag="b2")
            nc.sync.dma_start(b2_t[:], B2_d[:])
            wo_t = constp.tile([HID, ODIM], BF16, tag="wo")
            nc.sync.dma_start(wo_t[:], WO_d[:])
            bo_t = constp.tile([ODIM, 1], F32, tag="bo")
            nc.sync.dma_start(bo_t[:], BO_d[:])
            mskP = constp.tile([128, NT], F32, tag="mskP")
            nc.sync.dma_start(mskP[:], MSKP_d[:])

            at_t = atp.tile([128, 4 * NT * 512], BF16, tag="at")
            at5 = at_t[:].rearrange("p (i j c) -> p i j c", j=NT, c=512)
            for i in range(4):
                step = 4 if i == 0 else 8
                for j0 in range(0, NT, step):
                    nc.gpsimd.dma_start(
                        at5[:, i, j0 : j0 + step, :],
                        ATP_d[i, :, j0 : j0 + step, :],
                    )

            xp3 = xp_t[:].rearrange("p (t f) -> p t f", f=XF)
            hT1 = htp.tile([128, N], BF16, tag="ht", name="hT1")
            msg1 = msgp.tile([128, N], BF16, tag="msg", name="msg1")
            out_sb = xop.tile([128, NT * ODIM], F32, tag="xo", name="out_sb")

            def agg_mm(ap_ps, i, j, msg_nat, start, stop):
                nc.tensor.matmul(
                    ap_ps[0:HID, :],
                    msg_nat[:, j * 128 : (j + 1) * 128],
                    at5[:, i, j, :],
                    start=start,
                    stop=stop,
                )

            def emit_relu(l, i, ap_ps, hT_next):
                with nc.named_scope(f"relu{l}"):
                    dst = hT_next[:, i * 512 : (i + 1) * 512]
                    if i % 2 == 0:
                        nc.scalar.activation(dst, ap_ps[0:HID, :], Act.Relu)
                    else:
                        nc.vector.tensor_scalar_max(dst, ap_ps[0:HID, :], 0.0)

            Alu = mybir.AluOpType

            def emit_msg_mm(lname, i, q, hT, w_t, mp):
                with nc.named_scope(lname):
                    j = 4 * i + q
                    nc.tensor.matmul(
                        mp[:, q * 128 : (q + 1) * 128],
                        hT[:, j * 128 : (j + 1) * 128],
                        w_t[:],
                        start=True,
                        stop=True,
                    )

            def emit_msg_evac(lname, i, q, b_bc, mp, msg_nat):
                with nc.named_scope(lname):
                    j = 4 * i + q
                    nc.vector.scalar_tensor_tensor(
                        msg_nat[:, j * 128 : (j + 1) * 128],
                        mp[:, q * 128 : (q + 1) * 128],
                        1.0,
                        b_bc[:],
                        Alu.mult,
                        Alu.add,
                    )

            projT_tiles = {}

            def emit_projT(i, hT):
                with nc.named_scope("proj"):
                    pp = workp.tile([128, 512], F32, tag="workp",
                                    name=f"pp{i}")
                    nc.tensor.matmul(
                        pp[0:ODIM, :],
                        wo_t[:],
                        hT[:, i * 512 : (i + 1) * 512],
                        start=True,
                        stop=True,
                    )
                    projT = msgtp.tile([128, 512], BF16, tag="msgt",
                                       name=f"projT{i}")
                    if i % 2 == 0:
                        nc.scalar.activation(
                            projT[0:ODIM, :], pp[0:ODIM, :], Act.Identity,
                            bias=bo_t[:],
                        )
                    else:
                        nc.vector.tensor_scalar_add(
                            projT[0:ODIM, :], pp[0:ODIM, :], bo_t[:]
                        )
                    projT_tiles[i] = projT

            def emit_proj_out(i, eng=None):
                with nc.named_scope("proj"):
                    projT = projT_tiles.pop(i)
                    ps = workp.tile([128, 512], BF16, tag="workp",
                                    name=f"otp{i}")
                    for q in range(4):
                        nc.tensor.transpose(
                            ps[:, q * 64 : (q + 1) * 64],
                            projT[0:ODIM, q * 128 : (q + 1) * 128],
                            ident_b[0:ODIM, 0:ODIM],
                        )
                    for q in range(4):
                        j = 4 * i + q
                        if q % 2 == 0:
                            nc.scalar.activation(
                                out_sb[:, j * ODIM : (j + 1) * ODIM],
                                ps[:, q * 64 : (q + 1) * 64],
                                Act.Copy,
                                scale=mskP[:, j : j + 1],
                            )
                        else:
                            nc.vector.tensor_scalar_mul(
                                out_sb[:, j * ODIM : (j + 1) * ODIM],
                                ps[:, q * 64 : (q + 1) * 64],
                                mskP[:, j : j + 1],
                            )
                    (eng or nc.sync).dma_start(
                        Y3[:, 4 * i : 4 * i + 4, :],
                        out_sb[:].rearrange("p (t f) -> p t f", f=ODIM)[
                            :, 4 * i : 4 * i + 4, :
                        ],
                    )

            aps1 = {}
            done1 = {ip: set() for ip in range(4)}
            pending = []

            def agg1_step(ip, j):
                if ip not in aps1:
                    aps1[ip] = aggp.tile(
                        [128, 512], F32, tag="agg", name=f"agg1_{ip}"
                    )
                start = not done1[ip]
                done1[ip].add(j)
                agg_mm(aps1[ip], ip, j, msg1,
                       start=start, stop=(len(done1[ip]) == NT))

            def emit_partials(n, max_ip):
                with nc.named_scope("agg1"):
                    k, emitted = 0, 0
                    while k < len(pending) and emitted < n:
                        ip, j = pending[k]
                        if ip <= max_ip:
                            pending.pop(k)
                            agg1_step(ip, j)
                            emitted += 1
                        else:
                            k += 1

            msg2 = None
            hT2 = None
            for i in range(4):
                emit_partials(99, i - 1)
                with nc.named_scope("agg0"):
                    aht_ps = ahtpp.tile([128, 512], F32, tag="ahtps",
                                        name=f"aht{i}")
                    for j in range(NT):
                        nc.tensor.matmul(
                            aht_ps[0:XF, :],
                            xp3[:, j, :],
                            at5[:, i, j, :],
                            start=(j == 0),
                            stop=(j == NT - 1),
                        )
                    aht_sb = ahtsp.tile([128, 512], BF16, tag="aht",
                                        name=f"ahts{i}")
                    if i % 2 == 0:
                        nc.scalar.activation(
                            aht_sb[0:XF, :], aht_ps[0:XF, :], Act.Copy
                        )
                    else:
                        nc.vector.tensor_copy(aht_sb[0:XF, :], aht_ps[0:XF, :])
                emit_partials(4, i)
                with nc.named_scope("post0"):
                    post_ps = workp.tile([128, 512], F32, tag="workp",
                                         name=f"post{i}")
                    nc.tensor.matmul(
                        post_ps[0:HID, :],
                        w0p_t[:],
                        aht_sb[0:XF, :],
                        start=True,
                        stop=True,
                    )
                emit_partials(3, i)
                emit_relu(0, i, post_ps, hT1)
                emit_partials(3, i)
                m1ps = workp.tile([128, 512], F32, tag="workp",
                                  name=f"m1ps{i}")
                for q in range(4):
                    emit_msg_mm("msg1", i, q, hT1, w1_t, m1ps)
                    emit_msg_evac("msg1", i, q, b1_t, m1ps, msg1)
                    if q == 1:
                        emit_partials(2, i)
                for j in range(4 * i, 4 * i + 4):
                    for ip in range(4):
                        pending.append((ip, j))
                if i == 3:
                    msg2 = msgp.tile([128, N], BF16, tag="msg", name="msg2")
                    hT2 = htp.tile([128, N], BF16, tag="ht", name="hT2")
                    for ip in range(4):
                        with nc.named_scope("agg1"):
                            for (ipp, j) in [p for p in pending
                                             if p[0] == ip]:
                                pending.remove((ipp, j))
                                agg1_step(ip, j)
                        emit_relu(1, ip, aps1[ip], hT2)
                        m2ps = workp.tile([128, 512], F32, tag="workp",
                                          name=f"m2ps{ip}")
                        for q in range(4):
                            emit_msg_mm("msg2", ip, q, hT2, w2_t, m2ps)
                            emit_msg_evac("msg2", ip, q, b2_t, m2ps, msg2)

            hT3 = htp.tile([128, N], BF16, tag="ht", name="hT3")
            for i in range(4):
                ap_ps = aggp.tile([128, 512], F32, tag="agg",
                                  name=f"agg2_{i}")
                with nc.named_scope("agg2"):
                    for j in range(NT):
                        agg_mm(ap_ps, i, j, msg2,
                               start=(j == 0), stop=(j == NT - 1))
                if i < 3:
                    emit_relu(2, i, ap_ps, hT3)
                    if i >= 1:
                        emit_projT(i - 1, hT3)
                    if i >= 2:
                        emit_proj_out(i - 2)
            with nc.named_scope("relu2"):
                nc.scalar.activation(
                    hT3[:, 1536:1792], ap_ps[0:HID, 0:256], Act.Relu
                )
                nc.vector.tensor_scalar_max(
                    hT3[:, 1792:2048], ap_ps[0:HID, 256:512], 0.0
                )
            emit_projT(2, hT3)
            emit_proj_out(1)
            projT3 = msgtp.tile([128, 512], BF16, tag="msgt", name="projT3")
            with nc.named_scope("proj"):
                for h in range(2):
                    c0 = 1536 + h * 256
                    pp = workp.tile([128, 512], F32, tag="workp",
                                    name=f"pp3h{h}")
                    nc.tensor.matmul(
                        pp[0:ODIM, 0:256],
                        wo_t[:],
                        hT3[:, c0 : c0 + 256],
                        start=True,
                        stop=True,
                    )
                    if h == 0:
                        nc.scalar.activation(
                            projT3[0:ODIM, 0:256], pp[0:ODIM, 0:256],
                            Act.Identity, bias=bo_t[:],
                        )
                        emit_proj_out(2)
                    else:
                        nc.vector.tensor_scalar_add(
                            projT3[0:ODIM, 256:512], pp[0:ODIM, 0:256],
                            bo_t[:],
                        )
                for h in range(2):
                    ps = workp.tile([128, 512], BF16, tag="workp",
                                    name=f"otp3h{h}")
                    for qq in range(2):
                        q = 2 * h + qq
                        nc.tensor.transpose(
                            ps[:, q * 64 : (q + 1) * 64],
                            projT3[0:ODIM, q * 128 : (q + 1) * 128],
                            ident_b[0:ODIM, 0:ODIM],
                        )
                    for qq in range(2):
                        q = 2 * h + qq
                        j = 12 + q
                        if qq == 0:
                            nc.scalar.activation(
                                out_sb[:, j * ODIM : (j + 1) * ODIM],
                                ps[:, q * 64 : (q + 1) * 64],
                                Act.Copy,
                                scale=mskP[:, j : j + 1],
                            )
                        else:
                            nc.vector.tensor_scalar_mul(
                                out_sb[:, j * ODIM : (j + 1) * ODIM],
                                ps[:, q * 64 : (q + 1) * 64],
                                mskP[:, j : j + 1],
                            )
                    (nc.scalar if h == 0 else nc.gpsimd).dma_start(
                        Y3[:, 12 + 2 * h : 14 + 2 * h, :],
                        out_sb[:].rearrange("p (t f) -> p t f", f=ODIM)[
                            :, 12 + 2 * h : 14 + 2 * h, :
                        ],
                    )

    nc.compile()
    return nc


_EYE = np.eye(128, dtype=np.float32)


def _prep_v2(latent_features, adjacency_matrix, node_mask,
             W0, b0, W1, b1, W2, b2, Wout, bout):
    import ml_dtypes

    bf = ml_dtypes.bfloat16
    lat = np.asarray(latent_features, dtype=np.float32)
    adj = np.asarray(adjacency_matrix, dtype=np.float32)
    atp = np.ascontiguousarray(
        adj.reshape(B, 4, 512, NT, 128).transpose(0, 1, 4, 3, 2).astype(bf)
    )
    xa = np.concatenate(
        [lat, np.ones((B, N, 1), np.float32)], axis=2
    )
    xp = np.ascontiguousarray(
        xa.reshape(B, NT, 128, XF).transpose(0, 2, 1, 3).reshape(B, 128, NT * XF).astype(bf)
    )
    w0p = np.ascontiguousarray(
        np.concatenate(
            [np.asarray(W0, np.float32),
             np.asarray(b0, np.float32).reshape(1, HID)],
            axis=0,
        ).astype(bf)
    )
    w1b = np.ascontiguousarray(np.asarray(W1, np.float32).astype(bf))
    w2b = np.ascontiguousarray(np.asarray(W2, np.float32).astype(bf))
    wob = np.ascontiguousarray(np.asarray(Wout, np.float32).astype(bf))
    b1_ = np.ascontiguousarray(
        np.broadcast_to(np.asarray(b1, np.float32).reshape(1, HID),
                        (128, HID))
    )
    b2_ = np.ascontiguousarray(
        np.broadcast_to(np.asarray(b2, np.float32).reshape(1, HID),
                        (128, HID))
    )
    bo_ = np.asarray(bout, np.float32).reshape(ODIM, 1)
    msk = np.asarray(node_mask, np.float32).reshape(B, NT, 128)
    mskp = np.ascontiguousarray(msk.transpose(0, 2, 1))
    eye_b = np.ascontiguousarray(_EYE.astype(bf))

    in_maps = []
    for c in range(N_CORES):
        in_maps.append(
            {
                "ATP": atp[c],
                "XP": xp[c],
                "W0P": w0p,
                "W1": w1b,
                "B1": b1_,
                "W2": w2b,
                "B2": b2_,
                "WO": wob,
                "BO": bo_,
                "IDS": eye_b,
                "MSKP": mskp[c],
            }
        )
    return in_maps


def kernel(
    latent_features,
    adjacency_matrix,
    node_mask,
    W0,
    b0,
    W1,
    b1,
    W2,
    b2,
    Wout,
    bout,
    _trace=False,
    _agg_dt=None,
):
    nc = _build_v2()
    in_maps = _prep_v2(latent_features, adjacency_matrix, node_mask,
                       W0, b0, W1, b1, W2, b2, Wout, bout)
    res = run_bass_kernel_spmd(
        nc, in_maps, core_ids=list(range(N_CORES)), trace=_trace
    )
    out = np.stack([res.results[c]["Y"] for c in range(N_CORES)], axis=0)
    if _trace:
        return out, res
    return out


# revision 27
# speedup vs baseline: 1.2394x; 1.0067x over previous
import functools
import os

import numpy as np

import concourse.bass as bass
import concourse.bacc as bacc
import concourse.tile as tile
from concourse import mybir
from concourse.bass_utils import run_bass_kernel_spmd

B = 8
N = 2048
NT = N // 128
LAT = 64
XF = LAT + 1
HID = 128
ODIM = 64
N_CORES = 8

F32 = mybir.dt.float32
BF16 = mybir.dt.bfloat16
Act = mybir.ActivationFunctionType


@functools.lru_cache(maxsize=2)
def _build_v2():
    nc = bacc.Bacc(None, target_bir_lowering=False, debug=False)

    ATP_d = nc.declare_dram_parameter("ATP", [4, 128, NT, 512], BF16,
                                      isOutput=False)
    XP_d = nc.declare_dram_parameter("XP", [128, NT * XF], BF16,
                                     isOutput=False)
    W0P_d = nc.declare_dram_parameter("W0P", [XF, HID], BF16, isOutput=False)
    W1_d = nc.declare_dram_parameter("W1", [HID, HID], BF16, isOutput=False)
    W2_d = nc.declare_dram_parameter("W2", [HID, HID], BF16, isOutput=False)
    WO_d = nc.declare_dram_parameter("WO", [HID, ODIM], BF16, isOutput=False)
    B1_d = nc.declare_dram_parameter("B1", [128, HID], F32, isOutput=False)
    B2_d = nc.declare_dram_parameter("B2", [128, HID], F32, isOutput=False)
    BO_d = nc.declare_dram_parameter("BO", [ODIM, 1], F32, isOutput=False)
    IDS_d = nc.declare_dram_parameter("IDS", [128, 128], BF16, isOutput=False)
    MSKP_d = nc.declare_dram_parameter("MSKP", [128, NT], F32, isOutput=False)
    Y_d = nc.declare_dram_parameter("Y", [N, ODIM], F32, isOutput=True)
    Y3 = Y_d[:].rearrange("(t p) f -> p t f", p=128)

    with tile.TileContext(nc) as tc:
        with (
            tc.tile_pool(name="const", bufs=1) as constp,
            tc.tile_pool(name="at", bufs=1) as atp,
            tc.tile_pool(name="ht", bufs=2) as htp,
            tc.tile_pool(name="msg", bufs=2) as msgp,
            tc.tile_pool(name="msgt", bufs=2) as msgtp,
            tc.tile_pool(name="aht", bufs=2) as ahtsp,
            tc.tile_pool(name="xo", bufs=1) as xop,
            tc.tile_pool(name="ahtps", bufs=1, space=bass.MemorySpace.PSUM) as ahtpp,
            tc.tile_pool(name="aggp", bufs=4, space=bass.MemorySpace.PSUM) as aggp,
            tc.tile_pool(name="workp", bufs=3, space=bass.MemorySpace.PSUM) as workp,
        ):
            xp_t = constp.tile([128, NT * XF], BF16, tag="xp")
            nc.scalar.dma_start(xp_t[:], XP_d[:])
            w0p_t = constp.tile([XF, HID], BF16, tag="w0p")
            nc.scalar.dma_start(w0p_t[:], W0P_d[:])
            ident_b = constp.tile([128, 128], BF16, tag="idb")
            nc.scalar.dma_start(ident_b[:], IDS_d[:])
            w1_t = constp.tile([HID, HID], BF16, tag="w1")
            nc.scalar.dma_start(w1_t[:], W1_d[:])
            b1_t = constp.tile([128, HID], F32, tag="b1")
            nc.scalar.dma_start(b1_t[:], B1_d[:])
            w2_t = constp.tile([HID, HID], BF16, tag="w2")
            nc.sync.dma_start(w2_t[:], W2_d[:])
            b2_t = constp.tile([128, HID], F32, tag="b2")
            nc.sync.dma_start(b2_t[:], B2_d[:])
            wo_t = constp.tile([HID, ODIM], BF16, tag="wo")
            nc.sync.dma_start(wo_t[:], WO_d[:])
            bo_t = constp.tile([ODIM, 1], F32, tag="bo")
            nc.sync.dma_start(bo_t[:], BO_d[:])
            mskP = constp.tile([128, NT], F32, tag="mskP")
            nc.sync.dma_start(mskP[:], MSKP_d[:])

            at_t = atp.tile([128, 4 * NT * 512], BF16, tag="at")
            at5 = at_t[:].rearrange("p (i j c) -> p i j c", j=NT, c=512)
            for i in range(4):
                step = 4 if i == 0 else 8
                for j0 in range(0, NT, step):
                    nc.gpsimd.dma_start(
                        at5[:, i, j0 : j0 + step, :],
                        ATP_d[i, :, j0 : j0 + step, :],
                    )

            xp3 = xp_t[:].rearrange("p (t f) -> p t f", f=XF)
            hT1 = htp.tile([128, N], BF16, tag="ht", name="hT1")
            msg1 = msgp.tile([128, N], BF16, tag="msg", name="msg1")
            out_sb = xop.tile([128, NT * ODIM], F32, tag="xo", name="out_sb")

            def agg_mm(ap_ps, i, j, msg_nat, start, stop):
                nc.tensor.matmul(
                    ap_ps[0:HID, :],
                    msg_nat[:, j * 128 : (j + 1) * 128],
                    at5[:, i, j, :],
                    start=start,
                    stop=stop,
                )

            def emit_relu(l, i, ap_ps, hT_next):
                with nc.named_scope(f"relu{l}"):
                    dst = hT_next[:, i * 512 : (i + 1) * 512]
                    if i % 2 == 0:
                        nc.scalar.activation(dst, ap_ps[0:HID, :], Act.Relu)
                    else:
                        nc.vector.tensor_scalar_max(dst, ap_ps[0:HID, :], 0.0)

            Alu = mybir.AluOpType

            def emit_msg_mm(lname, i, q, hT, w_t, mp):
                with nc.named_scope(lname):
                    j = 4 * i + q
                    nc.tensor.matmul(
                        mp[:, q * 128 : (q + 1) * 128],
                        hT[:, j * 128 : (j + 1) * 128],
                        w_t[:],
                        start=True,
                        stop=True,
                    )

            def emit_msg_evac(lname, i, q, b_bc, mp, msg_nat):
                with nc.named_scope(lname):
                    j = 4 * i + q
                    nc.vector.scalar_tensor_tensor(
                        msg_nat[:, j * 128 : (j + 1) * 128],
                        mp[:, q * 128 : (q + 1) * 128],
                        1.0,
                        b_bc[:],
                        Alu.mult,
                        Alu.add,
                    )

            projT_tiles = {}

            def emit_projT(i, hT):
                with nc.named_scope("proj"):
                    pp = workp.tile([128, 512], F32, tag="workp",
                                    name=f"pp{i}")
                    nc.tensor.matmul(
                        pp[0:ODIM, :],
                        wo_t[:],
                        hT[:, i * 512 : (i + 1) * 512],
                        start=True,
                        stop=True,
                    )
                    projT = msgtp.tile([128, 512], BF16, tag="msgt",
                                       name=f"projT{i}")
                    if i % 2 == 0:
                        nc.scalar.activation(
                            projT[0:ODIM, :], pp[0:ODIM, :], Act.Identity,
                            bias=bo_t[:],
                        )
                    else:
                        nc.vector.tensor_scalar_add(
                            projT[0:ODIM, :], pp[0:ODIM, :], bo_t[:]
                        )
                    projT_tiles[i] = projT

            def emit_proj_out(i, eng=None):
                with nc.named_scope("proj"):
                    projT = projT_tiles.pop(i)
                    ps = workp.tile([128, 512], BF16, tag="workp",
                                    name=f"otp{i}")
                    for q in range(4):
                        nc.tensor.transpose(
                            ps[:, q * 64 : (q + 1) * 64],
                            projT[0:ODIM, q * 128 : (q + 1) * 128],
                            ident_b[0:ODIM, 0:ODIM],
                        )
                    for q in range(4):
                        j = 4 * i + q
                        if q % 2 == 0:
                            nc.scalar.activation(
                                out_sb[:, j * ODIM : (j + 1) * ODIM],
                                ps[:, q * 64 : (q + 1) * 64],
                                Act.Copy,
                                scale=mskP[:, j : j + 1],
                            )
                        else:
                            nc.vector.tensor_scalar_mul(
                                out_sb[:, j * ODIM : (j + 1) * ODIM],
                                ps[:, q * 64 : (q + 1) * 64],
                                mskP[:, j : j + 1],
                            )
                    (eng or nc.sync).dma_start(
                        Y3[:, 4 * i : 4 * i + 4, :],
                        out_sb[:].rearrange("p (t f) -> p t f", f=ODIM)[
                            :, 4 * i : 4 * i + 4, :
                        ],
                    )

            aps1 = {}
            done1 = {ip: set() for ip in range(4)}
            pending = []

            def agg1_step(ip, j):
                if ip not in aps1:
                    aps1[ip] = aggp.tile(
                        [128, 512], F32, tag="agg", name=f"agg1_{ip}"
                    )
                start = not done1[ip]
                done1[ip].add(j)
                agg_mm(aps1[ip], ip, j, msg1,
                       start=start, stop=(len(done1[ip]) == NT))

            def emit_partials(n, max_ip):
                with nc.named_scope("agg1"):
                    k, emitted = 0, 0
                    while k < len(pending) and emitted < n:
                        ip, j = pending[k]
                        if ip <= max_ip:
                            pending.pop(k)
                            agg1_step(ip, j)
                            emitted += 1
                        else:
                            k += 1

            msg2 = None
            hT2 = None
            for i in range(4):
                emit_partials(99, i - 1)
                with nc.named_scope("agg0"):
                    aht_ps = ahtpp.tile([128, 512], F32, tag="ahtps",
                                        name=f"aht{i}")
                    for j in range(NT):
                        nc.tensor.matmul(
                            aht_ps[0:XF, :],
                            xp3[:, j, :],
                            at5[:, i, j, :],
                            start=(j == 0),
                            stop=(j == NT - 1),
                        )
                    aht_sb = ahtsp.tile([128, 512], BF16, tag="aht",
                                        name=f"ahts{i}")
                    if i % 2 == 0:
                        nc.scalar.activation(
                            aht_sb[0:XF, :], aht_ps[0:XF, :], Act.Copy
                        )
                    else:
                        nc.vector.tensor_copy(aht_sb[0:XF, :], aht_ps[0:XF, :])
                emit_partials(4, i)
                with nc.named_scope("post0"):
                    post_ps = workp.tile([128, 512], F32, tag="workp",
                                         name=f"post{i}")
                    nc.tensor.matmul(
                        post_ps[0:HID, :],
                        w0p_t[:],
                        aht_sb[0:XF, :],
                        start=True,
                        stop=True,
                    )
                emit_partials(3, i)
                emit_relu(0, i, post_ps, hT1)
                emit_partials(3, i)
                m1ps = workp.tile([128, 512], F32, tag="workp",
                                  name=f"m1ps{i}")
                for q in range(4):
                    emit_msg_mm("msg1", i, q, hT1, w1_t, m1ps)
                    emit_msg_evac("msg1", i, q, b1_t, m1ps, msg1)
                    if q == 1:
                        emit_partials(2, i)
                for j in range(4 * i, 4 * i + 4):
                    for ip in range(4):
                        pending.append((ip, j))
                if i == 3:
                    msg2 = msgp.tile([128, N], BF16, tag="msg", name="msg2")
                    hT2 = htp.tile([128, N], BF16, tag="ht", name="hT2")

                    def drain1(ip):
                        with nc.named_scope("agg1"):
                            for (ipp, j) in [p for p in pending
                                             if p[0] == ip]:
                                pending.remove((ipp, j))
                                agg1_step(ip, j)

                    def msg2_chunk(ip):
                        m2ps = workp.tile([128, 512], F32, tag="workp",
                                          name=f"m2ps{ip}")
                        for q in range(4):
                            emit_msg_mm("msg2", ip, q, hT2, w2_t, m2ps)
                            emit_msg_evac("msg2", ip, q, b2_t, m2ps, msg2)

                    drain1(0)
                    emit_relu(1, 0, aps1[0], hT2)
                    drain1(1)
                    msg2_chunk(0)
                    emit_relu(1, 1, aps1[1], hT2)
                    drain1(2)
                    msg2_chunk(1)
                    emit_relu(1, 2, aps1[2], hT2)
                    drain1(3)
                    msg2_chunk(2)
                    emit_relu(1, 3, aps1[3], hT2)
                    msg2_chunk(3)

            hT3 = htp.tile([128, N], BF16, tag="ht", name="hT3")
            for i in range(4):
                ap_ps = aggp.tile([128, 512], F32, tag="agg",
                                  name=f"agg2_{i}")
                with nc.named_scope("agg2"):
                    for j in range(NT):
                        agg_mm(ap_ps, i, j, msg2,
                               start=(j == 0), stop=(j == NT - 1))
                if i < 3:
                    emit_relu(2, i, ap_ps, hT3)
                    if i >= 1:
                        emit_projT(i - 1, hT3)
                    if i >= 2:
                        emit_proj_out(i - 2)
            with nc.named_scope("relu2"):
                nc.scalar.activation(
                    hT3[:, 1536:1792], ap_ps[0:HID, 0:256], Act.Relu
                )
                nc.vector.tensor_scalar_max(
                    hT3[:, 1792:2048], ap_ps[0:HID, 256:512], 0.0
                )
            emit_projT(2, hT3)
            emit_proj_out(1)
            projT3 = msgtp.tile([128, 512], BF16, tag="msgt", name="projT3")
            with nc.named_scope("proj"):
                for h in range(2):
                    c0 = 1536 + h * 256
                    pp = workp.tile([128, 512], F32, tag="workp",
                                    name=f"pp3h{h}")
                    nc.tensor.matmul(
                        pp[0:ODIM, 0:256],
                        wo_t[:],
                        hT3[:, c0 : c0 + 256],
                        start=True,
                        stop=True,
                    )
                    if h == 0:
                        nc.scalar.activation(
                            projT3[0:ODIM, 0:256], pp[0:ODIM, 0:256],
                            Act.Identity, bias=bo_t[:],
                        )
                        emit_proj_out(2)
                    else:
                        nc.vector.tensor_scalar_add(
                            projT3[0:ODIM, 256:512], pp[0:ODIM, 0:256],
                            bo_t[:],
                        )
                out4 = out_sb[:].rearrange("p (t f) -> p t f", f=ODIM)
                ps = workp.tile([128, 512], BF16, tag="workp", name="otp3")
                for q in range(4):
                    nc.tensor.transpose(
                        ps[:, q * 64 : (q + 1) * 64],
                        projT3[0:ODIM, q * 128 : (q + 1) * 128],
                        ident_b[0:ODIM, 0:ODIM],
                    )
                    j = 12 + q
                    if q % 2 == 0:
                        nc.scalar.activation(
                            out_sb[:, j * ODIM : (j + 1) * ODIM],
                            ps[:, q * 64 : (q + 1) * 64],
                            Act.Copy,
                            scale=mskP[:, j : j + 1],
                        )
                    else:
                        nc.vector.tensor_scalar_mul(
                            out_sb[:, j * ODIM : (j + 1) * ODIM],
                            ps[:, q * 64 : (q + 1) * 64],
                            mskP[:, j : j + 1],
                        )
                nc.scalar.dma_start(Y3[:, 12:14, :], out4[:, 12:14, :])
                nc.sync.dma_start(Y3[:, 14:15, :], out4[:, 14:15, :])
                nc.gpsimd.dma_start(Y3[:, 15:16, :], out4[:, 15:16, :])

    nc.compile()
    return nc


_EYE = np.eye(128, dtype=np.float32)


def _prep_v2(latent_features, adjacency_matrix, node_mask,
             W0, b0, W1, b1, W2, b2, Wout, bout):
    import ml_dtypes

    bf = ml_dtypes.bfloat16
    lat = np.asarray(latent_features, dtype=np.float32)
    adj = np.asarray(adjacency_matrix, dtype=np.float32)
    atp = np.ascontiguousarray(
        adj.reshape(B, 4, 512, NT, 128).transpose(0, 1, 4, 3, 2).astype(bf)
    )
    xa = np.concatenate(
        [lat, np.ones((B, N, 1), np.float32)], axis=2
    )
    xp = np.ascontiguousarray(
        xa.reshape(B, NT, 128, XF).transpose(0, 2, 1, 3).reshape(B, 128, NT * XF).astype(bf)
    )
    w0p = np.ascontiguousarray(
        np.concatenate(
            [np.asarray(W0, np.float32),
             np.asarray(b0, np.float32).reshape(1, HID)],
            axis=0,
        ).astype(bf)
    )
    w1b = np.ascontiguousarray(np.asarray(W1, np.float32).astype(bf))
    w2b = np.ascontiguousarray(np.asarray(W2, np.float32).astype(bf))
    wob = np.ascontiguousarray(np.asarray(Wout, np.float32).astype(bf))
    b1_ = np.ascontiguousarray(
        np.broadcast_to(np.asarray(b1, np.float32).reshape(1, HID),
                        (128, HID))
    )
    b2_ = np.ascontiguousarray(
        np.broadcast_to(np.asarray(b2, np.float32).reshape(1, HID),
                        (128, HID))
    )
    bo_ = np.asarray(bout, np.float32).reshape(ODIM, 1)
    msk = np.asarray(node_mask, np.float32).reshape(B, NT, 128)
    mskp = np.ascontiguousarray(msk.transpose(0, 2, 1))
    eye_b = np.ascontiguousarray(_EYE.astype(bf))

    in_maps = []
    for c in range(N_CORES):
        in_maps.append(
            {
                "ATP": atp[c],
                "XP": xp[c],
                "W0P": w0p,
                "W1": w1b,
                "B1": b1_,
                "W2": w2b,
                "B2": b2_,
                "WO": wob,
                "BO": bo_,
                "IDS": eye_b,
                "MSKP": mskp[c],
            }
        )
    return in_maps


def kernel(
    latent_features,
    adjacency_matrix,
    node_mask,
    W0,
    b0,
    W1,
    b1,
    W2,
    b2,
    Wout,
    bout,
    _trace=False,
    _agg_dt=None,
):
    nc = _build_v2()
    in_maps = _prep_v2(latent_features, adjacency_matrix, node_mask,
                       W0, b0, W1, b1, W2, b2, Wout, bout)
    res = run_bass_kernel_spmd(
        nc, in_maps, core_ids=list(range(N_CORES)), trace=_trace
    )
    out = np.stack([res.results[c]["Y"] for c in range(N_CORES)], axis=0)
    if _trace:
        return out, res
    return out


# revision 30
# speedup vs baseline: 1.3233x; 1.0677x over previous
import functools

import numpy as np

import concourse.bass as bass
import concourse.bacc as bacc
import concourse.tile as tile
from concourse import mybir
from concourse.bass_utils import run_bass_kernel_spmd

B = 8
N = 2048
NT = N // 128
LAT = 64
XF = LAT + 1
HID = 128
ODIM = 64
N_CORES = 8

F32 = mybir.dt.float32
BF16 = mybir.dt.bfloat16
Act = mybir.ActivationFunctionType


@functools.lru_cache(maxsize=2)
def _build_v2():
    nc = bacc.Bacc(None, target_bir_lowering=False, debug=False)

    ATP_d = nc.declare_dram_parameter("ATP", [4, 128, NT, 512], BF16,
                                      isOutput=False)
    XP_d = nc.declare_dram_parameter("XP", [128, NT * XF], BF16,
                                     isOutput=False)
    W0P_d = nc.declare_dram_parameter("W0P", [XF, HID], BF16, isOutput=False)
    W1_d = nc.declare_dram_parameter("W1", [HID, HID], BF16, isOutput=False)
    W2_d = nc.declare_dram_parameter("W2", [HID, HID], BF16, isOutput=False)
    WO_d = nc.declare_dram_parameter("WO", [HID, ODIM], BF16, isOutput=False)
    B1_d = nc.declare_dram_parameter("B1", [128, HID], F32, isOutput=False)
    B2_d = nc.declare_dram_parameter("B2", [128, HID], F32, isOutput=False)
    BO_d = nc.declare_dram_parameter("BO", [ODIM, 1], F32, isOutput=False)
    YT_d = nc.declare_dram_parameter("YT", [ODIM, N], BF16, isOutput=True)

    with tile.TileContext(nc) as tc:
        with (
            tc.tile_pool(name="const", bufs=1) as constp,
            tc.tile_pool(name="at", bufs=1) as atp,
            tc.tile_pool(name="ht", bufs=2) as htp,
            tc.tile_pool(name="msg", bufs=2) as msgp,
            tc.tile_pool(name="msgt", bufs=2) as msgtp,
            tc.tile_pool(name="aht", bufs=2) as ahtsp,
            tc.tile_pool(name="ahtps", bufs=1, space=bass.MemorySpace.PSUM) as ahtpp,
            tc.tile_pool(name="aggp", bufs=4, space=bass.MemorySpace.PSUM) as aggp,
            tc.tile_pool(name="workp", bufs=3, space=bass.MemorySpace.PSUM) as workp,
        ):
            xp_t = constp.tile([128, NT * XF], BF16, tag="xp")
            nc.scalar.dma_start(xp_t[:], XP_d[:])
            w0p_t = constp.tile([XF, HID], BF16, tag="w0p")
            nc.scalar.dma_start(w0p_t[:], W0P_d[:])
            w1_t = constp.tile([HID, HID], BF16, tag="w1")
            nc.scalar.dma_start(w1_t[:], W1_d[:])
            b1_t = constp.tile([128, HID], F32, tag="b1")
            nc.scalar.dma_start(b1_t[:], B1_d[:])
            w2_t = constp.tile([HID, HID], BF16, tag="w2")
            nc.sync.dma_start(w2_t[:], W2_d[:])
            b2_t = constp.tile([128, HID], F32, tag="b2")
            nc.sync.dma_start(b2_t[:], B2_d[:])
            wo_t = constp.tile([HID, ODIM], BF16, tag="wo")
            nc.sync.dma_start(wo_t[:], WO_d[:])
            bo_t = constp.tile([ODIM, 1], F32, tag="bo")
            nc.sync.dma_start(bo_t[:], BO_d[:])

            at_t = atp.tile([128, 4 * NT * 512], BF16, tag="at")
            at5 = at_t[:].rearrange("p (i j c) -> p i j c", j=NT, c=512)
            for i in range(4):
                step = 4 if i == 0 else 8
                for j0 in range(0, NT, step):
                    nc.gpsimd.dma_start(
                        at5[:, i, j0 : j0 + step, :],
                        ATP_d[i, :, j0 : j0 + step, :],
                    )

            xp3 = xp_t[:].rearrange("p (t f) -> p t f", f=XF)
            hT1 = htp.tile([128, N], BF16, tag="ht", name="hT1")
            msg1 = msgp.tile([128, N], BF16, tag="msg", name="msg1")
            Alu = mybir.AluOpType

            def agg_mm(ap_ps, i, j, msg_nat, start, stop):
                nc.tensor.matmul(
                    ap_ps[0:HID, :],
                    msg_nat[:, j * 128 : (j + 1) * 128],
                    at5[:, i, j, :],
                    start=start,
                    stop=stop,
                )

            def emit_relu(l, i, ap_ps, hT_next):
                with nc.named_scope(f"relu{l}"):
                    dst = hT_next[:, i * 512 : (i + 1) * 512]
                    if i % 2 == 0:
                        nc.scalar.activation(dst, ap_ps[0:HID, :], Act.Relu)
                    else:
                        nc.vector.tensor_scalar_max(dst, ap_ps[0:HID, :], 0.0)

            def emit_msg_mm(lname, i, q, hT, w_t, mp):
                with nc.named_scope(lname):
                    j = 4 * i + q
                    nc.tensor.matmul(
                        mp[:, q * 128 : (q + 1) * 128],
                        hT[:, j * 128 : (j + 1) * 128],
                        w_t[:],
                        start=True,
                        stop=True,
                    )

            def emit_msg_evac(lname, i, q, b_bc, mp, msg_nat):
                with nc.named_scope(lname):
                    j = 4 * i + q
                    eng = nc.vector
                    eng.scalar_tensor_tensor(
                        msg_nat[:, j * 128 : (j + 1) * 128],
                        mp[:, q * 128 : (q + 1) * 128],
                        1.0,
                        b_bc[:],
                        Alu.mult,
                        Alu.add,
                    )

            def emit_projT(i, hT, eng=None):
                with nc.named_scope("proj"):
                    pp = workp.tile([128, 512], F32, tag="workp",
                                    name=f"pp{i}")
                    nc.tensor.matmul(
                        pp[0:ODIM, :],
                        wo_t[:],
                        hT[:, i * 512 : (i + 1) * 512],
                        start=True,
                        stop=True,
                    )
                    pt = msgtp.tile([128, 512], BF16, tag="msgt",
                                    name=f"projT{i}")
                    if i % 2 == 0:
                        nc.scalar.activation(
                            pt[0:ODIM, :], pp[0:ODIM, :], Act.Identity,
                            bias=bo_t[:],
                        )
                    else:
                        nc.vector.tensor_scalar_add(
                            pt[0:ODIM, :], pp[0:ODIM, :], bo_t[:]
                        )
                    (eng or nc.sync).dma_start(
                        YT_d[:, i * 512 : (i + 1) * 512], pt[0:ODIM, :]
                    )

            aps1 = {}
            done1 = {ip: set() for ip in range(4)}
            pending = []

            def agg1_step(ip, j):
                if ip not in aps1:
                    aps1[ip] = aggp.tile(
                        [128, 512], F32, tag="agg", name=f"agg1_{ip}"
                    )
                start = not done1[ip]
                done1[ip].add(j)
                agg_mm(aps1[ip], ip, j, msg1,
                       start=start, stop=(len(done1[ip]) == NT))

            def emit_partials(n, max_ip):
                with nc.named_scope("agg1"):
                    k, emitted = 0, 0
                    while k < len(pending) and emitted < n:
                        ip, j = pending[k]
                        if ip <= max_ip:
                            pending.pop(k)
                            agg1_step(ip, j)
                            emitted += 1
                        else:
                            k += 1

            msg2 = None
            hT2 = None
            for i in range(4):
                emit_partials(99, i - 1)
                with nc.named_scope("agg0"):
                    aht_ps = ahtpp.tile([128, 512], F32, tag="ahtps",
                                        name=f"aht{i}")
                    for j in range(NT):
                        nc.tensor.matmul(
                            aht_ps[0:XF, :],
                            xp3[:, j, :],
                            at5[:, i, j, :],
                            start=(j == 0),
                            stop=(j == NT - 1),
                        )
                    aht_sb = ahtsp.tile([128, 512], BF16, tag="aht",
                                        name=f"ahts{i}")
                    if i % 2 == 0:
                        nc.scalar.activation(
                            aht_sb[0:XF, :], aht_ps[0:XF, :], Act.Copy
                        )
                    else:
                        nc.vector.tensor_copy(aht_sb[0:XF, :], aht_ps[0:XF, :])
                emit_partials(4, i)
                with nc.named_scope("post0"):
                    post_ps = workp.tile([128, 512], F32, tag="workp",
                                         name=f"post{i}")
                    nc.tensor.matmul(
                        post_ps[0:HID, :],
                        w0p_t[:],
                        aht_sb[0:XF, :],
                        start=True,
                        stop=True,
                    )
                emit_partials(3, i)
                emit_relu(0, i, post_ps, hT1)
                emit_partials(3, i)
                m1ps = workp.tile([128, 512], F32, tag="workp",
                                  name=f"m1ps{i}")
                for q in range(4):
                    emit_msg_mm("msg1", i, q, hT1, w1_t, m1ps)
                    emit_msg_evac("msg1", i, q, b1_t, m1ps, msg1)
                    if q == 1:
                        emit_partials(2, i)
                for j in range(4 * i, 4 * i + 4):
                    for ip in range(4):
                        pending.append((ip, j))
                if i == 3:
                    msg2 = msgp.tile([128, N], BF16, tag="msg", name="msg2")
                    hT2 = htp.tile([128, N], BF16, tag="ht", name="hT2")

                    def drain1(ip):
                        with nc.named_scope("agg1"):
                            for (ipp, j) in [p for p in pending
                                             if p[0] == ip]:
                                pending.remove((ipp, j))
                                agg1_step(ip, j)

                    def msg2_chunk(ip):
                        m2ps = workp.tile([128, 512], F32, tag="workp",
                                          name=f"m2ps{ip}")
                        for q in range(4):
                            emit_msg_mm("msg2", ip, q, hT2, w2_t, m2ps)
                            emit_msg_evac("msg2", ip, q, b2_t, m2ps, msg2)

                    drain1(0)
                    emit_relu(1, 0, aps1[0], hT2)
                    drain1(1)
                    msg2_chunk(0)
                    emit_relu(1, 1, aps1[1], hT2)
                    drain1(2)
                    msg2_chunk(1)
                    emit_relu(1, 2, aps1[2], hT2)
                    drain1(3)
                    msg2_chunk(2)
                    emit_relu(1, 3, aps1[3], hT2)
                    msg2_chunk(3)

            hT3 = htp.tile([128, N], BF16, tag="ht", name="hT3")
            for i in range(4):
                ap_ps = aggp.tile([128, 512], F32, tag="agg",
                                  name=f"agg2_{i}")
                with nc.named_scope("agg2"):
                    for j in range(NT):
                        agg_mm(ap_ps, i, j, msg2,
                               start=(j == 0), stop=(j == NT - 1))
                if i < 3:
                    emit_relu(2, i, ap_ps, hT3)
                    if i >= 1:
                        emit_projT(i - 1, hT3,
                                   nc.sync if i == 1 else nc.scalar)
            with nc.named_scope("relu2"):
                nc.scalar.activation(
                    hT3[:, 1536:1792], ap_ps[0:HID, 0:256], Act.Relu
                )
                nc.vector.tensor_scalar_max(
                    hT3[:, 1792:2048], ap_ps[0:HID, 256:512], 0.0
                )
            emit_projT(2, hT3, nc.sync)
            with nc.named_scope("proj"):
                pt3 = msgtp.tile([128, 512], BF16, tag="msgt",
                                 name="projT3")
                for h in range(2):
                    c0 = 1536 + h * 256
                    pp = workp.tile([128, 512], F32, tag="workp",
                                    name=f"pp3h{h}")
                    nc.tensor.matmul(
                        pp[0:ODIM, 0:256],
                        wo_t[:],
                        hT3[:, c0 : c0 + 256],
                        start=True,
                        stop=True,
                    )
                    if h == 0:
                        nc.scalar.activation(
                            pt3[0:ODIM, 0:256], pp[0:ODIM, 0:256],
                            Act.Identity, bias=bo_t[:],
                        )
                        nc.scalar.dma_start(
                            YT_d[:, 1536:1792], pt3[0:ODIM, 0:256]
                        )
                    else:
                        nc.vector.tensor_scalar_add(
                            pt3[0:ODIM, 256:512], pp[0:ODIM, 0:256],
                            bo_t[:],
                        )
                        nc.gpsimd.dma_start(
                            YT_d[:, 1792:2048], pt3[0:ODIM, 256:512]
                        )

    nc.compile()
    return nc


def _prep_v2(latent_features, adjacency_matrix, node_mask,
             W0, b0, W1, b1, W2, b2, Wout, bout):
    import ml_dtypes

    bf = ml_dtypes.bfloat16
    lat = np.asarray(latent_features, dtype=np.float32)
    adj = np.asarray(adjacency_matrix, dtype=np.float32)
    atp = np.ascontiguousarray(
        adj.reshape(B, 4, 512, NT, 128).transpose(0, 1, 4, 3, 2).astype(bf)
    )
    xa = np.concatenate(
        [lat, np.ones((B, N, 1), np.float32)], axis=2
    )
    xp = np.ascontiguousarray(
        xa.reshape(B, NT, 128, XF).transpose(0, 2, 1, 3)
        .reshape(B, 128, NT * XF).astype(bf)
    )
    w0p = np.ascontiguousarray(
        np.concatenate(
            [np.asarray(W0, np.float32),
             np.asarray(b0, np.float32).reshape(1, HID)],
            axis=0,
        ).astype(bf)
    )
    w1b = np.ascontiguousarray(np.asarray(W1, np.float32).astype(bf))
    w2b = np.ascontiguousarray(np.asarray(W2, np.float32).astype(bf))
    wob = np.ascontiguousarray(np.asarray(Wout, np.float32).astype(bf))
    b1_ = np.ascontiguousarray(
        np.broadcast_to(np.asarray(b1, np.float32).reshape(1, HID),
                        (128, HID))
    )
    b2_ = np.ascontiguousarray(
        np.broadcast_to(np.asarray(b2, np.float32).reshape(1, HID),
                        (128, HID))
    )
    bo_ = np.asarray(bout, np.float32).reshape(ODIM, 1)

    in_maps = []
    for c in range(N_CORES):
        in_maps.append(
            {
                "ATP": atp[c],
                "XP": xp[c],
                "W0P": w0p,
                "W1": w1b,
                "B1": b1_,
                "W2": w2b,
                "B2": b2_,
                "WO": wob,
                "BO": bo_,
            }
        )
    return in_maps


def kernel(
    latent_features,
    adjacency_matrix,
    node_mask,
    W0,
    b0,
    W1,
    b1,
    W2,
    b2,
    Wout,
    bout,
    _trace=False,
    _agg_dt=None,
):
    nc = _build_v2()
    in_maps = _prep_v2(latent_features, adjacency_matrix, node_mask,
                       W0, b0, W1, b1, W2, b2, Wout, bout)
    res = run_bass_kernel_spmd(
        nc, in_maps, core_ids=list(range(N_CORES)), trace=_trace
    )
    msk = np.asarray(node_mask, dtype=np.float32)
    out = np.stack(
        [
            np.asarray(res.results[c]["YT"]).astype(np.float32).T
            for c in range(N_CORES)
        ],
        axis=0,
    ) * msk
    if _trace:
        return out, res
    return out
